# revision 17
# baseline (speedup 1.0000x reference)
"""Trainium2 Bass kernel for the CrossAttention problem (self-contained).

Strategy: shard the N=4096 query rows across 8 cores (512 rows/core, both
batch elements). Everything is computed in transposed layout (features on
partitions, query rows on the free dim) so every matmul has a wide moving
operand.

v4 structure:
 - Dummy 32B AllGather issued at t=0 (no data deps) absorbs the
   first-collective device barrier (~30us) concurrently with input DMA.
 - PE warm-up burst at t=0 keeps the HAM clock gate hot through the initial
   DMA window.
 - All DRAM inputs are host-packed to the exact SBUF layout ([128, chunk,
   free] contiguous) so input DMAs are descriptor-light.
 - Phase A computes only the collective's prerequisites (cond-half qT,
   branch-0 kT, branch-0 QK, per-partition max of exp(sim)) and posts the
   real AllGather.  Branch-0 sims are parked as f16 exp(sim); the mask later
   applies multiplicatively: exp(s + w*ae) == exp(s) * exp(w*ae).  The
   gathered quantity is max(exp(sim)) per partition; ln() recovers the global
   max-sim in the tail.
 - Phase B input DMAs are ordering-gated behind the collective-input DMA so
   its descriptors never queue behind megabytes of phase-B traffic.
 - Phase B: remaining projections, full uc group (softmax via post-PV column
   scale), cond branches 1..3 with PV partials parked in SBUF (f16), uc
   output projection interleaved between cond pairs as PE filler.
 - Tail: wmask bcast, branch-0 softmax, PV(branch 0) + parked partials merged
   via an identity matmul accumulate (no DVE adds), cond output projection.
 - Softmax 1/Z per unit computed via DVE reciprocal or scalar-engine
   exp(-ln Z), attn multiply on DVE or GpSimd, per the static VARIANTS
   schedule (engine balance).
"""

import sys

sys.path.insert(0, "/opt/trn_rl_repo")

import numpy as np

import concourse.bass as bass
import concourse.tile as tile
from concourse import bacc, bass_utils, mybir

# problem constants (hardcoded per the harness contract)
H, DH, L, C = 8, 64, 77, 4
N, DQ, DC, INNER = 4096, 320, 768, 512
N_CORES = 8
NS = N // N_CORES          # query rows per core per batch element
NSB = 2 * NS               # both batch elements
SCALE = DH ** -0.5
W_DOT, TOTAL_STEP, SCHED = 1.0, 50, 4.6

F32 = mybir.dt.float32
F16 = mybir.dt.float16
EXP = mybir.ActivationFunctionType.Exp
LN = mybir.ActivationFunctionType.Ln
AXX = mybir.AxisListType.X

G_UC = 0

LAST_RESULTS = None  # BassKernelResults of the most recent run (for test.py)
TRACE = False

N_WARMUP_MM = 20

# normalization variant per unit: 'rd' recip + DVE mul, 'rg' recip + gpsimd
# mul, 'ld' lnexp + DVE mul, 'lg' lnexp + gpsimd mul.
COND_VAR = {
    (0, 2): 'rd', (0, 3): 'rg', (0, 4): 'rg',
    (1, 2): 'lg', (1, 3): 'rd', (1, 4): 'rg',
    (2, 2): 'rg', (2, 3): 'lg', (2, 4): 'rd',
    (3, 2): 'rg', (3, 3): 'rg', (3, 4): 'lg',
}
TAIL_VAR = {0: 'rd', 1: 'rg', 2: 'rd', 3: 'rg'}


def _gate(a, b, reason="order"):
    if a is not None and b is not None:
        tile.add_dep_helper(a.ins, b.ins, sync=False, reason=reason)


def build_kernel(wdotw: float):
    nc = bacc.Bacc("TRN2", target_bir_lowering=False, debug=False, num_devices=N_CORES)

    d_xtc = nc.dram_tensor("xtc", [128, 3, NS], F16, kind="ExternalInput")
    d_xtu = nc.dram_tensor("xtu", [128, 3, NS], F16, kind="ExternalInput")
    d_wq = nc.dram_tensor("wq", [128, 3, INNER], F16, kind="ExternalInput")
    d_wk = nc.dram_tensor("wk", [128, 6, INNER], F16, kind="ExternalInput")
    d_wv = nc.dram_tensor("wv", [128, 6, INNER], F16, kind="ExternalInput")
    d_wo = nc.dram_tensor("wo", [128, 4, DQ], F16, kind="ExternalInput")
    d_bo = nc.dram_tensor("bo", [128, 3], F32, kind="ExternalInput")
    d_k0 = nc.dram_tensor("k0", [128, 6, L], F16, kind="ExternalInput")
    d_kr = nc.dram_tensor("kr", [128, 6, 4 * L], F16, kind="ExternalInput")
    d_vt = nc.dram_tensor("vt", [128, 6, 5 * L], F16, kind="ExternalInput")
    d_aet = nc.dram_tensor("aet", [L, H, NS], F16, kind="ExternalInput")
    d_id = nc.dram_tensor("idm", [128, 64], F16, kind="ExternalInput")
    d_yt = nc.dram_tensor("yt", [DQ, NSB], F16, kind="ExternalOutput")

    with tile.TileContext(nc) as tc:
        _emit(nc, tc, wdotw, d_xtc, d_xtu, d_wq, d_wk, d_wv, d_wo, d_bo,
              d_k0, d_kr, d_vt, d_aet, d_id, d_yt)
    nc.compile()
    return nc


def _emit(nc, tc, wdotw, d_xtc, d_xtu, d_wq, d_wk, d_wv, d_wo, d_bo,
          d_k0, d_kr, d_vt, d_aet, d_id, d_yt):
    from contextlib import ExitStack

    ctx = ExitStack()
    singles = ctx.enter_context(tc.tile_pool(name="singles", bufs=1))
    dram = ctx.enter_context(tc.tile_pool(name="dram", bufs=1, space="DRAM"))
    epool = ctx.enter_context(tc.tile_pool(name="epool", bufs=8))
    apool = ctx.enter_context(tc.tile_pool(name="apool", bufs=8))
    rzpool = ctx.enter_context(tc.tile_pool(name="rzpool", bufs=4))
    lnpool = ctx.enter_context(tc.tile_pool(name="lnpool", bufs=4))
    psim = ctx.enter_context(tc.tile_pool(name="psim", bufs=2, space="PSUM"))

    # ---- persistent SBUF tiles ----
    s_xtc = singles.tile([128, 3, NS], F16)
    s_xtu = singles.tile([128, 3, NS], F16)
    s_wq = singles.tile([128, 3, INNER], F16)
    s_wk = singles.tile([128, 6, INNER], F16)
    s_wv = singles.tile([128, 6, INNER], F16)
    s_wo = singles.tile([128, 4, DQ], F16)
    s_bo = singles.tile([128, 3], F32)
    s_k0c = singles.tile([128, 6, L], F16)
    s_krc = singles.tile([128, 6, 4 * L], F16)
    s_vtc = singles.tile([128, 6, 5 * L], F16)
    s_aet = singles.tile([L, H, NS], F16)
    s_qt = singles.tile([128, 4, NSB], F16)
    s_kt = singles.tile([128, 4, 4 * L], F16)   # groups: uc, c1, c2, c3
    s_ktc0 = singles.tile([128, 4, L], F16)
    s_vp = singles.tile([L, 5, INNER], F16)
    s_e0 = singles.tile([L, H, NS], F16)        # exp(branch-0 sims), pre-mask
    s_pvp = singles.tile([128, 4, NS], F16)     # cond PV partials (br 1..3)
    s_om = singles.tile([128, 4, NSB], F16)
    s_y = singles.tile([128, 3, NSB], F16)
    s_id = singles.tile([128, 64], F16)
    s_lmax = singles.tile([L, H], F32)
    s_lm = singles.tile([L, 1], F32)
    s_maxrow8 = singles.tile([1, N_CORES * L], F32)
    s_wme = singles.tile([1, 1], F32)
    s_wml = singles.tile([1, 1], F16)
    s_wmcol = singles.tile([L, 1], F32)
    ones77 = singles.tile([L, L], F16)
    ones_row = singles.tile([1, 128], F16)
    junk = singles.tile([128, INNER], F16)

    # ---- dummy collective: absorb the first-collective device barrier.
    # Reads an unwritten DRAM scratch tile so it has no data dependencies.
    din = dram.tile([1, 8], F32)
    dout = dram.tile([N_CORES, 8], F32)
    dummy_cc = nc.gpsimd.collective_compute(
        "AllGather", mybir.AluOpType.bypass,
        replica_groups=[list(range(N_CORES))],
        ins=[din.opt()], outs=[dout.opt()],
    )

    # ---- PE warm-up burst ----
    nc.vector.memset(junk[:], 0.03)
    pproj_cm = tc.tile_pool(name="pproj", bufs=2, space="PSUM")
    pproj = pproj_cm.__enter__()
    pjunk_cm = tc.tile_pool(name="pjunk", bufs=1, space="PSUM")
    pjunk = pjunk_cm.__enter__()
    pj = pjunk.tile([128, NS], F32)
    for _ in range(N_WARMUP_MM):
        nc.tensor.matmul(pj[:], junk[:, 0:128], junk[:, 0:NS], start=True, stop=True)

    # ---- phase A input DMA (lean: only the collective path's inputs) ----
    nc.sync.dma_start(out=s_wq[:], in_=d_wq.ap())
    nc.sync.dma_start(out=s_xtc[:], in_=d_xtc.ap())
    nc.sync.dma_start(out=s_wk[:], in_=d_wk.ap())
    nc.sync.dma_start(out=s_k0c[:], in_=d_k0.ap())

    nc.vector.memset(ones77[:], 1.0)
    nc.vector.memset(ones_row[:], 1.0)

    def qproj(half, s_x):
        for dc in range(4):
            p = pproj.tile([128, NS], F32, tag="proj")
            for kc in range(3):
                nc.tensor.matmul(
                    p[:],
                    s_wq[:, kc, dc * 128:(dc + 1) * 128],
                    s_x[:, kc, :],
                    start=(kc == 0), stop=(kc == 2),
                )
            nc.scalar.copy(s_qt[:, dc, half * NS:(half + 1) * NS], p[:])

    # ---- phase A: just enough for the branch-0 max -> collective ----
    qproj(1, s_xtc)                              # cond-half qT
    for dc in range(4):                          # branch-0 kT
        p = pproj.tile([128, L], F32, tag="proj")
        for kc in range(6):
            nc.tensor.matmul(
                p[:],
                s_wk[:, kc, dc * 128:(dc + 1) * 128],
                s_k0c[:, kc, :],
                start=(kc == 0), stop=(kc == 5),
            )
        nc.scalar.copy(s_ktc0[:, dc, :], p[:])

    def qk0(h, psum_slice):
        nc.tensor.matmul(
            psum_slice,
            s_ktc0[(h % 2) * 64:(h % 2) * 64 + 64, h // 2, :],
            s_qt[(h % 2) * 64:(h % 2) * 64 + 64, h // 2, NS:NSB],
            start=True, stop=True,
        )

    for hp in range(4):
        p = psim.tile([L, 2, NS], F32, tag="sim")
        qk0(2 * hp, p[:, 0, :])
        qk0(2 * hp + 1, p[:, 1, :])
        nc.scalar.activation(s_e0[:, 2 * hp:2 * hp + 2, :], p[:], EXP)
        # per-partition max of exp(sim0) (monotonic in sim; ln in the tail)
        nc.vector.reduce_max(out=s_lmax[:, 2 * hp:2 * hp + 2],
                             in_=s_e0[:, 2 * hp:2 * hp + 2, :], axis=AXX)
    nc.vector.reduce_max(out=s_lm[:], in_=s_lmax[:], axis=AXX)

    cin = dram.tile([1, L], F32)
    cout = dram.tile([N_CORES, L], F32)
    cin_dma = nc.sync.dma_start(out=cin.rearrange("one f -> f one"), in_=s_lm[:])
    real_cc = nc.gpsimd.collective_compute(
        "AllGather", mybir.AluOpType.bypass,
        replica_groups=[list(range(N_CORES))],
        ins=[cin.opt()], outs=[cout.opt()],
    )
    _gate(real_cc, dummy_cc, "real collective after dummy")
    pjunk_cm.__exit__(None, None, None)

    # ---- phase B input DMA (issue-gated behind the collective input) ----
    for dd, ss in ((d_xtu, s_xtu), (d_wv, s_wv), (d_vt, s_vtc), (d_wo, s_wo),
                   (d_bo, s_bo), (d_kr, s_krc), (d_id, s_id), (d_aet, s_aet)):
        dm = nc.sync.dma_start(out=ss[:], in_=dd.ap())
        _gate(dm, cin_dma, "phase B DMA after collective input DMA")
    # tail input: gathered per-core branch-0 maxima (waits on the collective)
    nc.sync.dma_start(out=s_maxrow8[:], in_=cout.rearrange("r f -> (r f)"))

    # ---- phase B: remaining projections ----
    qproj(0, s_xtu)                              # uc-half qT
    for dc in range(4):                          # kT for groups uc,c1,c2,c3
        p = pproj.tile([128, 4 * L], F32, tag="proj")
        for kc in range(6):
            nc.tensor.matmul(
                p[:],
                s_wk[:, kc, dc * 128:(dc + 1) * 128],
                s_krc[:, kc, :],
                start=(kc == 0), stop=(kc == 5),
            )
        nc.vector.tensor_copy(s_kt[:, dc, :], p[:])
    for g in range(5):                           # v, with 1/C folded into cond
        p = pproj.tile([128, INNER], F32, tag="proj")
        for kc in range(6):
            nc.tensor.matmul(
                p[0:L, :],
                s_vtc[:, kc, g * L:(g + 1) * L],
                s_wv[:, kc, :],
                start=(kc == 0), stop=(kc == 5),
            )
        if g == G_UC:
            nc.vector.tensor_copy(s_vp[:, g, :], p[0:L, :])
        else:
            nc.vector.tensor_scalar_mul(s_vp[:, g, :], p[0:L, :], 1.0 / C)
    pproj_cm.__exit__(None, None, None)

    pzb_cm = tc.tile_pool(name="pzb", bufs=1, space="PSUM")
    pzb = pzb_cm.__enter__()
    ppv_cm = tc.tile_pool(name="ppv", bufs=2, space="PSUM")
    ppv = ppv_cm.__enter__()

    def qk(g, h, psum_slice):
        # s_kt slots: 0=uc, 1..3 = cond branches 1..3 (groups 2..4)
        gi = 0 if g == G_UC else g - 1
        cols = slice(0, NS) if g == G_UC else slice(NS, NSB)
        nc.tensor.matmul(
            psum_slice,
            s_kt[(h % 2) * 64:(h % 2) * 64 + 64, h // 2, gi * L:(gi + 1) * L],
            s_qt[(h % 2) * 64:(h % 2) * 64 + 64, h // 2, cols],
            start=True, stop=True,
        )

    def zmm(zb, e):
        # replicated per-column sums over the 77 key rows
        nc.tensor.matmul(zb[:, 0, :], ones77[:], e[:, 0, :], start=True, stop=True)
        nc.tensor.matmul(zb[:, 1, :], ones77[:], e[:, 1, :], start=True, stop=True)

    anchors = {"cc": real_cc}

    def normalize(e, zb, var):
        """Return attn tile a = e * (1/Z) [L,2,NS] f16 per the variant."""
        if var[0] == 'r':
            rz = rzpool.tile([L, 2, NS], F32, tag="rz")
            nc.vector.reciprocal_approx_fast(out=rz[:], in_=zb[:])
            rzs = rz[:]
        else:
            lnt = lnpool.tile([L, 2, NS], F32, tag="ln")
            nc.scalar.activation(lnt[:], zb[:], LN)
            rz = rzpool.tile([L, 2, NS], F16, tag="rzh")
            nc.scalar.activation(rz[:], lnt[:], EXP, scale=-1.0)
            rzs = rz[:]
        a = apool.tile([L, 2, NS], F16, tag="attn")
        eng = nc.vector if var[1] == 'd' else nc.gpsimd
        mi = eng.tensor_mul(a[:], e[:], rzs)
        if var[1] == 'g':
            _gate(mi, anchors.get("cc"), "gpsimd mul after collective trigger")
        return a

    def pv_head(pv, g, h, a_slice, start, stop):
        return nc.tensor.matmul(pv[:], s_vp[:, g, h * 64:(h + 1) * 64],
                                a_slice, start=start, stop=stop)

    def wo_oc(half, oc):
        ow = 128 if oc < 2 else 64
        p = ppv.tile([128, NS], F32, tag="pv")
        for kc in range(4):
            nc.tensor.matmul(
                p[0:ow, :],
                s_wo[:, kc, oc * 128:oc * 128 + ow],
                s_om[:, kc, half * NS:(half + 1) * NS],
                start=(kc == 0), stop=(kc == 3),
            )
        nc.scalar.add(s_y[0:ow, oc, half * NS:(half + 1) * NS], p[0:ow, :],
                      s_bo[0:ow, oc:oc + 1])

    def y_dma(half):
        for oc in range(3):
            ow = 128 if oc < 2 else 64
            nc.sync.dma_start(
                out=d_yt.ap()[oc * 128:oc * 128 + ow, half * NS:(half + 1) * NS],
                in_=s_y[0:ow, oc, half * NS:(half + 1) * NS])

    # ---- phase B: uc group (QK, softmax via post-PV column scaling) ----
    for hp in range(4):
        p = psim.tile([L, 2, NS], F32, tag="sim")
        qk(0, 2 * hp, p[:, 0, :])
        qk(0, 2 * hp + 1, p[:, 1, :])
        e = epool.tile([L, 2, NS], F16, tag="e")
        nc.scalar.activation(e[:], p[:], EXP)
        zb = pzb.tile([L, 2, NS], F32, tag="zb")
        zmm(zb, e)
        rz = rzpool.tile([L, 2, NS], F32, tag="rz")
        nc.vector.reciprocal_approx_fast(out=rz[:], in_=zb[:])
        for k in range(2):
            h = 2 * hp + k
            pv = ppv.tile([64, NS], F32, tag="pv")
            pv_head(pv, 0, h, e[:, k, :], True, True)
            nc.vector.tensor_mul(s_om[(h % 2) * 64:(h % 2) * 64 + 64, hp, 0:NS],
                                 pv[:], rz[0:64, k, :])

    # ---- cond branches 1..3 (groups 2,3,4) pair-major, wo(0) interleaved ----
    for hp in range(4):
        atiles = {}
        for g in (2, 3, 4):
            p = psim.tile([L, 2, NS], F32, tag="sim")
            qk(g, 2 * hp, p[:, 0, :])
            qk(g, 2 * hp + 1, p[:, 1, :])
            e = epool.tile([L, 2, NS], F16, tag="e")
            anchors["act"] = nc.scalar.activation(e[:], p[:], EXP)
            zb = pzb.tile([L, 2, NS], F32, tag="zb")
            zmm(zb, e)
            atiles[g] = normalize(e, zb, COND_VAR[(hp, g)])
        for k in range(2):
            h = 2 * hp + k
            pv = ppv.tile([64, NS], F32, tag="pv")
            for i, g in enumerate((2, 3, 4)):
                anchors["mm"] = pv_head(pv, g, h, atiles[g][:, k, :],
                                        i == 0, i == 2)
            anchors["dve"] = nc.vector.tensor_copy(
                s_pvp[(h % 2) * 64:(h % 2) * 64 + 64, hp, :], pv[:])
        atiles.clear()
        if hp < 3:
            wo_oc(0, hp)    # uc output projection chunks fill PE gaps
    y_dma(0)

    # ---- tail: wmask, branch 0, PV merge via identity-matmul ----
    red = nc.vector.reduce_max(out=s_wme[:], in_=s_maxrow8[:], axis=AXX)
    _gate(red, anchors.get("dve"), "defer wmask reduce behind phase B")
    lni = nc.scalar.activation(s_wml[:], s_wme[:], LN)
    p_wm = ppv.tile([128, NS], F32, tag="pv")
    bc = nc.tensor.matmul(p_wm[0:L, 0:1], ones_row[0:1, 0:L], s_wml[:],
                          start=True, stop=True)
    _gate(bc, anchors.get("mm"), "defer wmask bcast behind phase B matmuls")
    wmc = nc.scalar.mul(s_wmcol[:], p_wm[0:L, 0:1], float(wdotw))
    _gate(lni, anchors.get("act"), "defer wmask ln behind phase B exps")

    first = {}
    for hp in range(4):
        m = epool.tile([L, 2, NS], F16, tag="e")
        mex = nc.scalar.activation(m[:], s_aet[:, 2 * hp:2 * hp + 2, :], EXP,
                                   scale=s_wmcol[:])
        first.setdefault("act", mex)
        e0m = epool.tile([L, 2, NS], F16, tag="e")
        mm0 = nc.vector.tensor_mul(e0m[:], s_e0[:, 2 * hp:2 * hp + 2, :], m[:])
        first.setdefault("dve", mm0)
        zb = pzb.tile([L, 2, NS], F32, tag="zb")
        zmm(zb, e0m)
        a = normalize(e0m, zb, TAIL_VAR[hp])
        for k in range(2):
            h = 2 * hp + k
            rows = slice((h % 2) * 64, (h % 2) * 64 + 64)
            pv = ppv.tile([64, NS], F32, tag="pv")
            pv_head(pv, 1, h, a[:, k, :], True, False)
            # merge the parked branch 1..3 partial via identity accumulate
            nc.tensor.matmul(pv[:], s_id[rows, :], s_pvp[rows, hp, :],
                             start=False, stop=True)
            nc.scalar.copy(s_om[rows, hp, NS:NSB], pv[:])
    _gate(first.get("act"), anchors.get("act"), "tail ACT after phase B ACT")
    _gate(first.get("dve"), anchors.get("dve"), "tail DVE after phase B DVE")

    # ---- cond half of the output projection ----
    for oc in range(3):
        wo_oc(1, oc)
    y_dma(1)

    ppv_cm.__exit__(None, None, None)
    pzb_cm.__exit__(None, None, None)
    ctx.pop_all().close()


_CACHE = {}


def _packP(a, p=128):
    rows, cols = a.shape
    pad = (-rows) % p
    if pad:
        a = np.vstack([a, np.zeros((pad, cols), a.dtype)])
    c = a.shape[0] // p
    return np.ascontiguousarray(a.reshape(c, p, cols).transpose(1, 0, 2))


def kernel(x, uc_context, ck, cv, attn_extra, Wq, Wk, Wv, Wo, bo, t):
    global LAST_RESULTS
    x = np.ascontiguousarray(np.asarray(x, np.float32))
    uc_context = np.asarray(uc_context, np.float32)
    ck = np.asarray(ck, np.float32)
    cv = np.asarray(cv, np.float32)
    attn_extra = np.asarray(attn_extra, np.float32)
    Wq = np.asarray(Wq, np.float32)
    Wk = np.asarray(Wk, np.float32)
    Wv = np.asarray(Wv, np.float32)
    Wo = np.asarray(Wo, np.float32)
    bo = np.asarray(bo, np.float32)
    tv = float(np.asarray(t))
    wdotw = W_DOT * (tv / TOTAL_STEP) * SCHED

    if wdotw not in _CACHE:
        _CACHE[wdotw] = build_kernel(wdotw)
    nc = _CACHE[wdotw]

    # host-side input prep (layout only)
    wq_pad = np.zeros((384, INNER), np.float16)
    wq_pad[:DQ] = (Wq * SCALE).astype(np.float16)
    bo_pad = np.zeros((384,), np.float32)
    bo_pad[:DQ] = bo
    wqp = _packP(wq_pad)
    wkp = _packP(Wk.astype(np.float16))
    wvp = _packP(Wv.astype(np.float16))
    wop = _packP(Wo.astype(np.float16))
    bop = np.ascontiguousarray(bo_pad.reshape(3, 128).T)
    ctxK = np.concatenate([uc_context[0][None], ck[:, 0]], axis=0)  # [5,77,768]
    ctxV = np.concatenate([uc_context[0][None], cv[:, 0]], axis=0)
    ctxkt = ctxK.transpose(2, 0, 1).reshape(DC, 5 * L).astype(np.float16)
    ctxvt = ctxV.transpose(2, 0, 1).reshape(DC, 5 * L).astype(np.float16)
    k0p = _packP(np.ascontiguousarray(ctxkt[:, L:2 * L]))
    krp = _packP(np.ascontiguousarray(
        np.concatenate([ctxkt[:, 0:L], ctxkt[:, 2 * L:]], axis=1)))
    vtp = _packP(ctxvt)
    idm = np.vstack([np.eye(64, dtype=np.float16)] * 2)
    idm = np.ascontiguousarray(idm)

    in_maps = []
    for c in range(N_CORES):
        rows = slice(c * NS, (c + 1) * NS)
        xtu = _packP(np.vstack([x[0, rows].T.astype(np.float16),
                                np.zeros((64, NS), np.float16)]))
        xtc = _packP(np.vstack([x[1, rows].T.astype(np.float16),
                                np.zeros((64, NS), np.float16)]))
        aet = np.ascontiguousarray(
            attn_extra[:, rows, :].transpose(2, 0, 1)).astype(np.float16)
        in_maps.append({
            "xtc": xtc, "xtu": xtu, "wq": wqp, "wk": wkp, "wv": wvp,
            "wo": wop, "bo": bop, "k0": k0p, "kr": krp, "vt": vtp,
            "aet": aet, "idm": idm,
        })

    import os as _os
    _tc = None
    if _os.environ.get("KERNEL_TRACE_ALL") == "1":
        _tc = list(range(N_CORES))
    res = bass_utils.run_bass_kernel_spmd(
        nc, in_maps, core_ids=list(range(N_CORES)), trace=TRACE, trace_cores=_tc,
    )
    LAST_RESULTS = res

    out = np.empty((2, N, DQ), np.float32)
    for c in range(N_CORES):
        rows = slice(c * NS, (c + 1) * NS)
        yt = res.results[c]["yt"]
        out[0, rows] = yt[:, :NS].T.astype(np.float32)
        out[1, rows] = yt[:, NS:].T.astype(np.float32)
    return out


# revision 18
# speedup vs baseline: 1.2165x; 1.2165x over previous
"""Trainium2 Bass kernel for the CrossAttention problem (self-contained).

Strategy: shard the N=4096 query rows across 8 cores (512 rows/core, both
batch elements). Everything is computed in transposed layout (features on
partitions, query rows on the free dim) so every matmul has a wide moving
operand:

  qT   = (scale*Wq)^T @ xT          [512, 1024]   (rows 0:512 uc, 512:1024 cond)
  kT   = Wk^T @ ctxKT               [512, 5*77]   (uc, c0..c3 contexts)
  v    = ctxVT_g^T @ Wv             [5][77, 512]
  simT = k_gh @ qT_h                [77, 512] per (group, head)
  E    = exp(simT)  (logits are small; no max-subtraction needed)
  Z    = ones^T @ E                 [1, 512] rows into a dense PSUM stack
  attn = E * bcast(1/Z)             (PE broadcast of the recip row)
  outT = v_gh^T @ attn  (accumulated over the 4 cond branches; uc separate)
  yT   = Wo^T @ out_mergedT + bo    [320, 1024]

The soft-mask scalar wmask = w_dot * (t/50*4.6) * max(sim_c[0]) couples all
cores: each core computes its local branch-0 max, an AllGather collective
combines them while the other 4 groups are processed, then branch 0 finishes.

v5 deltas over the v1 baseline:
 - A dummy 32B AllGather with no data deps is issued at t=0 so the expensive
   first-collective device barrier overlaps input DMA / early compute.
 - All DRAM inputs are host-packed to the exact SBUF layout ([128, chunk,
   free], contiguous) so input DMAs are single descriptor-light transfers.
 - The PV accumulation for cond branches 1..3 happens at the end of phase 3
   (partials parked in SBUF as f16); the post-collective tail only adds the
   branch-0 PV via matmul and merges the partial with an identity-matmul
   accumulate, halving the tail's PE work.
 - The output is written as f16 (tolerance is 2e-2; f16 rounding is ~5e-4).
"""

import sys

sys.path.insert(0, "/opt/trn_rl_repo")

import numpy as np

import concourse.bass as bass
import concourse.tile as tile
from concourse import bacc, bass_utils, mybir

# problem constants (hardcoded per the harness contract)
H, DH, L, C = 8, 64, 77, 4
N, DQ, DC, INNER = 4096, 320, 768, 512
N_CORES = 8
NS = N // N_CORES          # query rows per core per batch element
NSB = 2 * NS               # both batch elements
SCALE = DH ** -0.5
W_DOT, TOTAL_STEP, SCHED = 1.0, 50, 4.6

F32 = mybir.dt.float32
F16 = mybir.dt.float16

# groups in context order: 0=uc, 1..4 = cond branches 0..3
G_UC = 0

LAST_RESULTS = None  # BassKernelResults of the most recent run (for test.py)
TRACE = False


def build_kernel(wdotw: float):
    nc = bacc.Bacc("TRN2", target_bir_lowering=False, debug=False, num_devices=N_CORES)

    # ---- DRAM I/O (host-packed to SBUF layout) ----
    d_xt = nc.dram_tensor("xt", [128, 3, NSB], F16, kind="ExternalInput")
    d_wq = nc.dram_tensor("wq", [128, 3, INNER], F16, kind="ExternalInput")  # pre-scaled
    d_wk = nc.dram_tensor("wk", [128, 6, INNER], F16, kind="ExternalInput")
    d_wv = nc.dram_tensor("wv", [128, 6, INNER], F16, kind="ExternalInput")
    d_wo = nc.dram_tensor("wo", [128, 4, DQ], F16, kind="ExternalInput")
    d_bo = nc.dram_tensor("bo", [128, 3], F32, kind="ExternalInput")
    d_ctxkt = nc.dram_tensor("ctxkt", [128, 6, 5 * L], F16, kind="ExternalInput")
    d_ctxvt = nc.dram_tensor("ctxvt", [128, 6, 5 * L], F16, kind="ExternalInput")
    d_aet = nc.dram_tensor("aet", [H, L, NS], F32, kind="ExternalInput")
    d_id = nc.dram_tensor("idm", [128, 64], F16, kind="ExternalInput")
    d_yt = nc.dram_tensor("yt", [DQ, NSB], F16, kind="ExternalOutput")

    with tile.TileContext(nc) as tc:
        _emit(nc, tc, wdotw, d_xt, d_wq, d_wk, d_wv, d_wo, d_bo,
              d_ctxkt, d_ctxvt, d_aet, d_id, d_yt)
    nc.compile()
    return nc


def _emit(nc, tc, wdotw, d_xt, d_wq, d_wk, d_wv, d_wo, d_bo,
          d_ctxkt, d_ctxvt, d_aet, d_id, d_yt):
    from contextlib import ExitStack

    ctx = ExitStack()
    singles = ctx.enter_context(tc.tile_pool(name="singles", bufs=1))
    dram = ctx.enter_context(tc.tile_pool(name="dram", bufs=1, space="DRAM"))
    work = ctx.enter_context(tc.tile_pool(name="work", bufs=3))
    epool = ctx.enter_context(tc.tile_pool(name="epool", bufs=10))
    apool = ctx.enter_context(tc.tile_pool(name="apool", bufs=26))
    rzpool = ctx.enter_context(tc.tile_pool(name="rzpool", bufs=4))

    # ---- persistent SBUF tiles ----
    s_xt = singles.tile([128, 3, NSB], F16)
    s_wq = singles.tile([128, 3, INNER], F16)
    s_wk = singles.tile([128, 6, INNER], F16)
    s_wv = singles.tile([128, 6, INNER], F16)
    s_wo = singles.tile([128, 4, DQ], F16)
    s_bo = singles.tile([128, 3], F32)
    s_ctxkt = singles.tile([128, 6, 5 * L], F16)
    s_ctxvt = singles.tile([128, 6, 5 * L], F16)
    s_aet = singles.tile([L, H, NS], F32)
    s_qt = singles.tile([128, 4, NSB], F16)
    s_kt = singles.tile([128, 4, 5 * L], F16)
    s_ktc0 = singles.tile([128, 4, L], F16)
    s_vp = singles.tile([L, 5, INNER], F16)
    s_sc0 = singles.tile([L, H, NS], F32)       # branch-0 sims parked pre-mask
    s_pvp = singles.tile([128, 4, NS], F16)     # cond PV partials (branches 1..3)
    s_om = singles.tile([128, 4, NSB], F16)     # merged outT (inner on partitions)
    s_y = singles.tile([128, 3, NSB], F16)
    s_id = singles.tile([128, 64], F16)
    s_lmax = singles.tile([L, H], F32)
    s_lm = singles.tile([L, 1], F32)
    s_maxrow8 = singles.tile([1, N_CORES * L], F32)
    s_wm = singles.tile([1, 1], F16)
    s_wmcol = singles.tile([L, 1], F32)
    ones77 = singles.tile([L, L], F16)
    ones_row = singles.tile([1, 128], F16)

    # ---- dummy collective: absorb the first-collective device barrier ----
    din = dram.tile([1, 8], F32)
    dout = dram.tile([N_CORES, 8], F32)
    dummy_cc = nc.gpsimd.collective_compute(
        "AllGather", mybir.AluOpType.bypass,
        replica_groups=[list(range(N_CORES))],
        ins=[din.opt()], outs=[dout.opt()],
    )

    # ---- critical-path input DMA (collective prerequisites first) ----
    nc.sync.dma_start(out=s_xt[:], in_=d_xt.ap())
    nc.sync.dma_start(out=s_wq[:], in_=d_wq.ap())
    nc.sync.dma_start(out=s_wk[:], in_=d_wk.ap())
    nc.sync.dma_start(out=s_ctxkt[:], in_=d_ctxkt.ap())

    nc.vector.memset(ones77[:], 1.0)
    nc.vector.memset(ones_row[:], 1.0)

    psim = ctx.enter_context(tc.tile_pool(name="psim", bufs=2, space="PSUM"))
    pproj_cm = tc.tile_pool(name="pproj", bufs=2, space="PSUM")
    pproj = pproj_cm.__enter__()

    def qproj(half):
        for dc in range(4):
            p = pproj.tile([128, NS], F32, tag="proj")
            for kc in range(3):
                nc.tensor.matmul(
                    p[:],
                    s_wq[:, kc, dc * 128:(dc + 1) * 128],
                    s_xt[:, kc, half * NS:(half + 1) * NS],
                    start=(kc == 0), stop=(kc == 2),
                )
            nc.scalar.copy(s_qt[:, dc, half * NS:(half + 1) * NS], p[:])

    # ---- phase 0: just enough for the branch-0 max -> collective ----
    qproj(1)                                     # cond-half qT
    for dc in range(4):                          # branch-0 kT slice
        p = pproj.tile([128, 5 * L], F32, tag="proj")
        for kc in range(6):
            nc.tensor.matmul(
                p[0:128, 0:L],
                s_wk[:, kc, dc * 128:(dc + 1) * 128],
                s_ctxkt[:, kc, L:2 * L],
                start=(kc == 0), stop=(kc == 5),
            )
        nc.scalar.copy(s_ktc0[:, dc, :], p[0:128, 0:L])

    def qk0(h, psum_slice):
        nc.tensor.matmul(
            psum_slice,
            s_ktc0[(h % 2) * 64:(h % 2) * 64 + 64, h // 2, :],
            s_qt[(h % 2) * 64:(h % 2) * 64 + 64, h // 2, NS:NSB],
            start=True, stop=True,
        )

    for hp in range(4):
        p = psim.tile([L, 2, NS], F32, tag="sim")
        qk0(2 * hp, p[:, 0, :])
        qk0(2 * hp + 1, p[:, 1, :])
        nc.vector.reduce_max(out=s_lmax[:, 2 * hp:2 * hp + 2], in_=p[:],
                             axis=mybir.AxisListType.X)
        nc.scalar.copy(s_sc0[:, 2 * hp:2 * hp + 2, :], p[:])
    nc.vector.reduce_max(out=s_lm[:], in_=s_lmax[:], axis=mybir.AxisListType.X)
    nc.vector.tensor_scalar_mul(s_lm[:], s_lm[:], float(wdotw))

    cin = dram.tile([1, L], F32)
    cout = dram.tile([N_CORES, L], F32)
    nc.sync.dma_start(out=cin.rearrange("one f -> f one"), in_=s_lm[:])
    real_cc = nc.gpsimd.collective_compute(
        "AllGather", mybir.AluOpType.bypass,
        replica_groups=[list(range(N_CORES))],
        ins=[cin.opt()], outs=[cout.opt()],
    )
    tile.add_dep_helper(real_cc.ins, dummy_cc.ins, sync=False,
                        reason="real collective after dummy")

    # ---- remaining input DMA ----
    nc.sync.dma_start(out=s_wv[:], in_=d_wv.ap())
    nc.sync.dma_start(out=s_ctxvt[:], in_=d_ctxvt.ap())
    nc.sync.dma_start(out=s_wo[:], in_=d_wo.ap())
    nc.sync.dma_start(out=s_bo[:], in_=d_bo.ap())
    nc.sync.dma_start(out=s_id[:], in_=d_id.ap())
    nc.sync.dma_start(out=s_aet[:], in_=d_aet.ap().rearrange("h p f -> p h f"))

    # ---- phase 1: remaining projections ----
    qproj(0)                                     # uc-half qT
    for dc in range(4):                          # full kT (branch-0 cols unused)
        p = pproj.tile([128, 5 * L], F32, tag="proj")
        for kc in range(6):
            nc.tensor.matmul(
                p[:],
                s_wk[:, kc, dc * 128:(dc + 1) * 128],
                s_ctxkt[:, kc, :],
                start=(kc == 0), stop=(kc == 5),
            )
        nc.scalar.copy(s_kt[:, dc, :], p[:])
    for g in range(5):                           # v, with 1/C folded into cond
        p = pproj.tile([128, INNER], F32, tag="proj")
        for kc in range(6):
            nc.tensor.matmul(
                p[0:L, :],
                s_ctxvt[:, kc, g * L:(g + 1) * L],
                s_wv[:, kc, :],
                start=(kc == 0), stop=(kc == 5),
            )
        if g == G_UC:
            nc.scalar.copy(s_vp[:, g, :], p[0:L, :])
        else:
            nc.scalar.mul(s_vp[:, g, :], p[0:L, :], 1.0 / C)
    pproj_cm.__exit__(None, None, None)

    pzb = ctx.enter_context(tc.tile_pool(name="pzb", bufs=1, space="PSUM"))
    ppv = ctx.enter_context(tc.tile_pool(name="ppv", bufs=2, space="PSUM"))

    def qk(g, h, psum_slice):
        cols = slice(0, NS) if g == G_UC else slice(NS, NSB)
        nc.tensor.matmul(
            psum_slice,
            s_kt[(h % 2) * 64:(h % 2) * 64 + 64, h // 2, g * L:(g + 1) * L],
            s_qt[(h % 2) * 64:(h % 2) * 64 + 64, h // 2, cols],
            start=True, stop=True,
        )

    # ---- phase 3: uc + branches 1..3 (unit pairs) ----
    attn_c = {}
    anchors = {}

    def unit_pair(e_pair):
        zb = pzb.tile([L, 2, NS], F32, tag="zb")
        nc.tensor.matmul(zb[:, 0, :], ones77[:], e_pair[:, 0, :], start=True, stop=True)
        anchors["zb"] = nc.tensor.matmul(
            zb[:, 1, :], ones77[:], e_pair[:, 1, :], start=True, stop=True)
        rz = rzpool.tile([L, 2, NS], F32, tag="rz")
        nc.vector.reciprocal_approx_fast(out=rz[:], in_=zb[:])
        return rz

    for g in (0, 2, 3, 4):
        sims = []
        for hp in range(4):
            p = psim.tile([L, 2, NS], F32, tag="sim")
            qk(g, 2 * hp, p[:, 0, :])
            qk(g, 2 * hp + 1, p[:, 1, :])
            sims.append(p)
        for hp in range(4):
            e = epool.tile([L, 2, NS], F16, tag="e")
            anchors["exp"] = nc.scalar.activation(
                e[:], sims[hp][:], mybir.ActivationFunctionType.Exp)
            rz = unit_pair(e)
            for k in range(2):
                h = 2 * hp + k
                if g == G_UC:
                    pv = ppv.tile([64, NS], F32, tag="pv")
                    nc.tensor.matmul(pv[:], s_vp[:, 0, h * 64:(h + 1) * 64],
                                     e[:, k, :], start=True, stop=True)
                    nc.vector.tensor_mul(
                        s_om[(h % 2) * 64:(h % 2) * 64 + 64, h // 2, 0:NS],
                        pv[:], rz[0:64, k, :])
                else:
                    a = apool.tile([L, NS], F16, tag="attn")
                    anchors["mul"] = nc.vector.tensor_mul(a[:], e[:, k, :], rz[:, k, :])
                    attn_c[(g, h)] = a

    # ---- pre-accumulate the cond PV partials for branches 1..3 ----
    for h in range(H):
        pv = ppv.tile([64, NS], F32, tag="pv")
        for i, g in enumerate((2, 3, 4)):
            nc.tensor.matmul(pv[:], s_vp[:, g, h * 64:(h + 1) * 64],
                             attn_c.pop((g, h))[:], start=(i == 0), stop=(i == 2))
        anchors["evac"] = nc.vector.tensor_copy(
            s_pvp[(h % 2) * 64:(h % 2) * 64 + 64, h // 2, :], pv[:])

    # ---- uc half of the output projection (independent of branch 0) ----
    def wo_half(half, pool):
        for oc in range(3):
            ow = 128 if oc < 2 else 64
            p = pool.tile([128, NS], F32, tag="pv")
            for kc in range(4):
                nc.tensor.matmul(
                    p[0:ow, :],
                    s_wo[:, kc, oc * 128:oc * 128 + ow],
                    s_om[:, kc, half * NS:(half + 1) * NS],
                    start=(kc == 0), stop=(kc == 3),
                )
            nc.scalar.add(s_y[0:ow, oc, half * NS:(half + 1) * NS], p[0:ow, :],
                          s_bo[0:ow, oc:oc + 1])
        for oc in range(3):
            ow = 128 if oc < 2 else 64
            nc.sync.dma_start(
                out=d_yt.ap()[oc * 128:oc * 128 + ow, half * NS:(half + 1) * NS],
                in_=s_y[0:ow, oc, half * NS:(half + 1) * NS])

    wo_half(0, ppv)

    # ---- phase 4: wmask from the gathered maxima, branch 0, PV finish ----
    nc.sync.dma_start(out=s_maxrow8[:], in_=cout.rearrange("r f -> (r f)"))
    red = nc.vector.reduce_max(out=s_wm[:], in_=s_maxrow8[:], axis=mybir.AxisListType.X)
    tile.add_dep_helper(red.ins, anchors["mul"].ins, sync=False,
                        reason="defer wmask path behind group work")
    p_wm = pzb.tile([L, 2, NS], F32, tag="zb")
    bc = nc.tensor.matmul(p_wm[:, 0, 0:1], ones_row[0:1, 0:L], s_wm[:],
                          start=True, stop=True)
    tile.add_dep_helper(bc.ins, anchors["zb"].ins, sync=False,
                        reason="defer wmask bcast behind group matmuls")
    nc.vector.tensor_copy(s_wmcol[:], p_wm[:, 0, 0:1])

    first_p4_exp = None
    for hp in range(4):
        msk = work.tile([L, 2, NS], F32, tag="msk")
        nc.vector.scalar_tensor_tensor(
            out=msk[:], in0=s_aet[:, 2 * hp:2 * hp + 2, :], scalar=s_wmcol[:],
            in1=s_sc0[:, 2 * hp:2 * hp + 2, :],
            op0=mybir.AluOpType.mult, op1=mybir.AluOpType.add,
        )
        e = epool.tile([L, 2, NS], F16, tag="e")
        ei = nc.scalar.activation(e[:], msk[:], mybir.ActivationFunctionType.Exp)
        if first_p4_exp is None:
            first_p4_exp = ei
            tile.add_dep_helper(ei.ins, anchors["exp"].ins, sync=False,
                                reason="defer branch-0 exp behind group exps")
        rz = unit_pair(e)
        for k in range(2):
            h = 2 * hp + k
            a = apool.tile([L, NS], F16, tag="attn")
            nc.vector.tensor_mul(a[:], e[:, k, :], rz[:, k, :])
            rows = slice((h % 2) * 64, (h % 2) * 64 + 64)
            pv = ppv.tile([64, NS], F32, tag="pv")
            nc.tensor.matmul(pv[:], s_vp[:, 1, h * 64:(h + 1) * 64],
                             a[:], start=True, stop=False)
            # merge the parked branch 1..3 partial via identity accumulate
            nc.tensor.matmul(pv[:], s_id[rows, :], s_pvp[rows, h // 2, :],
                             start=False, stop=True)
            nc.scalar.copy(s_om[rows, h // 2, NS:NSB], pv[:])

    # ---- phase 5: cond half of the output projection ----
    wo_half(1, ppv)
    ctx.pop_all().close()


_CACHE = {}


def _packP(a, p=128):
    rows, cols = a.shape
    pad = (-rows) % p
    if pad:
        a = np.vstack([a, np.zeros((pad, cols), a.dtype)])
    c = a.shape[0] // p
    return np.ascontiguousarray(a.reshape(c, p, cols).transpose(1, 0, 2))


def kernel(x, uc_context, ck, cv, attn_extra, Wq, Wk, Wv, Wo, bo, t):
    global LAST_RESULTS
    x = np.ascontiguousarray(np.asarray(x, np.float32))
    uc_context = np.asarray(uc_context, np.float32)
    ck = np.asarray(ck, np.float32)
    cv = np.asarray(cv, np.float32)
    attn_extra = np.asarray(attn_extra, np.float32)
    Wq = np.asarray(Wq, np.float32)
    Wk = np.asarray(Wk, np.float32)
    Wv = np.asarray(Wv, np.float32)
    Wo = np.asarray(Wo, np.float32)
    bo = np.asarray(bo, np.float32)
    tv = float(np.asarray(t))
    wdotw = W_DOT * (tv / TOTAL_STEP) * SCHED

    if wdotw not in _CACHE:
        _CACHE[wdotw] = build_kernel(wdotw)
    nc = _CACHE[wdotw]

    # host-side input prep (layout only)
    wq_pad = np.zeros((384, INNER), np.float16)
    wq_pad[:DQ] = (Wq * SCALE).astype(np.float16)
    bo_pad = np.zeros((384,), np.float32)
    bo_pad[:DQ] = bo
    wqp = _packP(wq_pad)
    wkp = _packP(Wk.astype(np.float16))
    wvp = _packP(Wv.astype(np.float16))
    wop = _packP(Wo.astype(np.float16))
    bop = np.ascontiguousarray(bo_pad.reshape(3, 128).T)
    ctxK = np.concatenate([uc_context[0][None], ck[:, 0]], axis=0)  # [5, 77, 768]
    ctxV = np.concatenate([uc_context[0][None], cv[:, 0]], axis=0)
    ctxkt = _packP(np.ascontiguousarray(
        ctxK.transpose(2, 0, 1).reshape(DC, 5 * L)).astype(np.float16))
    ctxvt = _packP(np.ascontiguousarray(
        ctxV.transpose(2, 0, 1).reshape(DC, 5 * L)).astype(np.float16))
    idm = np.ascontiguousarray(np.vstack([np.eye(64, dtype=np.float16)] * 2))

    in_maps = []
    for c in range(N_CORES):
        rows = slice(c * NS, (c + 1) * NS)
        xt = np.zeros((384, NSB), np.float16)
        xt[:DQ, :NS] = x[0, rows].T.astype(np.float16)
        xt[:DQ, NS:] = x[1, rows].T.astype(np.float16)
        aet = np.ascontiguousarray(attn_extra[:, rows, :].transpose(0, 2, 1))
        in_maps.append({
            "xt": _packP(xt), "wq": wqp, "wk": wkp, "wv": wvp, "wo": wop,
            "bo": bop, "ctxkt": ctxkt, "ctxvt": ctxvt, "aet": aet, "idm": idm,
        })

    import os as _os
    _tc = None
    if _os.environ.get("KERNEL_TRACE_ALL") == "1":
        _tc = list(range(N_CORES))
    res = bass_utils.run_bass_kernel_spmd(
        nc, in_maps, core_ids=list(range(N_CORES)), trace=TRACE, trace_cores=_tc,
    )
    LAST_RESULTS = res

    out = np.empty((2, N, DQ), np.float32)
    for c in range(N_CORES):
        rows = slice(c * NS, (c + 1) * NS)
        yt = res.results[c]["yt"]
        out[0, rows] = yt[:, :NS].T.astype(np.float32)
        out[1, rows] = yt[:, NS:].T.astype(np.float32)
    return out


# revision 21
# speedup vs baseline: 1.2760x; 1.0489x over previous
"""Trainium2 Bass kernel for the CrossAttention problem (self-contained).

Strategy: shard the N=4096 query rows across 8 cores (512 rows/core, both
batch elements). Everything is computed in transposed layout (features on
partitions, query rows on the free dim) so every matmul has a wide moving
operand:

  qT   = (scale*Wq)^T @ xT          [512, 1024]   (rows 0:512 uc, 512:1024 cond)
  kT   = Wk^T @ ctxKT               [512, 5*77]   (uc, c0..c3 contexts)
  v    = ctxVT_g^T @ Wv             [5][77, 512]
  simT = k_gh @ qT_h                [77, 512] per (group, head)
  E    = exp(simT)  (logits are small; no max-subtraction needed)
  Z    = ones^T @ E                 [1, 512] rows into a dense PSUM stack
  attn = E * bcast(1/Z)             (PE broadcast of the recip row)
  outT = v_gh^T @ attn  (accumulated over the 4 cond branches; uc separate)
  yT   = Wo^T @ out_mergedT + bo    [320, 1024]

The soft-mask scalar wmask = w_dot * (t/50*4.6) * max(sim_c[0]) couples all
cores: each core computes its local branch-0 max, an AllGather collective
combines them while the other 4 groups are processed, then branch 0 finishes.

v5 deltas over the v1 baseline:
 - A dummy 32B AllGather with no data deps is issued at t=0 so the expensive
   first-collective device barrier overlaps input DMA / early compute.
 - All DRAM inputs are host-packed to the exact SBUF layout ([128, chunk,
   free], contiguous) so input DMAs are single descriptor-light transfers.
 - The PV accumulation for cond branches 1..3 happens at the end of phase 3
   (partials parked in SBUF as f16); the post-collective tail only adds the
   branch-0 PV via matmul and merges the partial with an identity-matmul
   accumulate, halving the tail's PE work.
 - The output is written as f16 (tolerance is 2e-2; f16 rounding is ~5e-4).
"""

import sys

sys.path.insert(0, "/opt/trn_rl_repo")

import numpy as np

import concourse.bass as bass
import concourse.tile as tile
from concourse import bacc, bass_utils, mybir

# problem constants (hardcoded per the harness contract)
H, DH, L, C = 8, 64, 77, 4
N, DQ, DC, INNER = 4096, 320, 768, 512
N_CORES = 8
NS = N // N_CORES          # query rows per core per batch element
NSB = 2 * NS               # both batch elements
SCALE = DH ** -0.5
W_DOT, TOTAL_STEP, SCHED = 1.0, 50, 4.6

F32 = mybir.dt.float32
F16 = mybir.dt.float16

# groups in context order: 0=uc, 1..4 = cond branches 0..3
G_UC = 0

LAST_RESULTS = None  # BassKernelResults of the most recent run (for test.py)
TRACE = False


def build_kernel(wdotw: float):
    nc = bacc.Bacc("TRN2", target_bir_lowering=False, debug=False, num_devices=N_CORES)

    # ---- DRAM I/O (host-packed to SBUF layout) ----
    d_xt = nc.dram_tensor("xt", [128, 3, NSB], F16, kind="ExternalInput")
    d_wq = nc.dram_tensor("wq", [128, 3, INNER], F16, kind="ExternalInput")  # pre-scaled
    d_wk = nc.dram_tensor("wk", [128, 6, INNER], F16, kind="ExternalInput")
    d_wv = nc.dram_tensor("wv", [128, 6, INNER], F16, kind="ExternalInput")
    d_wo = nc.dram_tensor("wo", [128, 4, DQ], F16, kind="ExternalInput")
    d_bo = nc.dram_tensor("bo", [128, 3], F32, kind="ExternalInput")
    d_ctxkt = nc.dram_tensor("ctxkt", [128, 6, 5 * L], F16, kind="ExternalInput")
    d_ctxvt = nc.dram_tensor("ctxvt", [128, 6, 5 * L], F16, kind="ExternalInput")
    d_aet = nc.dram_tensor("aet", [H, L, NS], F32, kind="ExternalInput")
    d_id = nc.dram_tensor("idm", [128, 64], F16, kind="ExternalInput")
    d_yt = nc.dram_tensor("yt", [DQ, NSB], F16, kind="ExternalOutput")

    with tile.TileContext(nc) as tc:
        _emit(nc, tc, wdotw, d_xt, d_wq, d_wk, d_wv, d_wo, d_bo,
              d_ctxkt, d_ctxvt, d_aet, d_id, d_yt)
    nc.compile()
    return nc


def _emit(nc, tc, wdotw, d_xt, d_wq, d_wk, d_wv, d_wo, d_bo,
          d_ctxkt, d_ctxvt, d_aet, d_id, d_yt):
    from contextlib import ExitStack

    ctx = ExitStack()
    singles = ctx.enter_context(tc.tile_pool(name="singles", bufs=1))
    dram = ctx.enter_context(tc.tile_pool(name="dram", bufs=1, space="DRAM"))
    work = ctx.enter_context(tc.tile_pool(name="work", bufs=3))
    epool = ctx.enter_context(tc.tile_pool(name="epool", bufs=10))
    apool = ctx.enter_context(tc.tile_pool(name="apool", bufs=26))
    rzpool = ctx.enter_context(tc.tile_pool(name="rzpool", bufs=4))

    # ---- persistent SBUF tiles ----
    s_xt = singles.tile([128, 3, NSB], F16)
    s_wq = singles.tile([128, 3, INNER], F16)
    s_wk = singles.tile([128, 6, INNER], F16)
    s_wv = singles.tile([128, 6, INNER], F16)
    s_wo = singles.tile([128, 4, DQ], F16)
    s_bo = singles.tile([128, 3], F32)
    s_ctxkt = singles.tile([128, 6, 5 * L], F16)
    s_ctxvt = singles.tile([128, 6, 5 * L], F16)
    s_aet = singles.tile([L, H, NS], F32)
    s_qt = singles.tile([128, 4, NSB], F16)
    s_kt = singles.tile([128, 4, 5 * L], F16)
    s_ktc0 = singles.tile([128, 4, L], F16)
    s_vp = singles.tile([L, 5, INNER], F16)
    s_sc0 = singles.tile([L, H, NS], F32)       # branch-0 sims parked pre-mask
    s_pvp = singles.tile([128, 4, NS], F16)     # cond PV partials (branches 1..3)
    s_om = singles.tile([128, 4, NSB], F16)     # merged outT (inner on partitions)
    s_y = singles.tile([128, 3, NSB], F16)
    s_id = singles.tile([128, 64], F16)
    s_lmax = singles.tile([L, H], F32)
    s_lm = singles.tile([L, 1], F32)
    s_maxrow8 = singles.tile([1, N_CORES * L], F32)
    s_wm = singles.tile([1, 1], F16)
    s_wmcol = singles.tile([L, 1], F32)
    ones77 = singles.tile([L, L], F16)
    ones_row = singles.tile([1, 128], F16)

    # ---- critical-path input DMA (collective prerequisites first) ----
    nc.sync.dma_start(out=s_xt[:], in_=d_xt.ap())
    nc.sync.dma_start(out=s_wq[:], in_=d_wq.ap())
    nc.sync.dma_start(out=s_wk[:], in_=d_wk.ap())
    nc.sync.dma_start(out=s_ctxkt[:], in_=d_ctxkt.ap())

    nc.vector.memset(ones77[:], 1.0)
    nc.vector.memset(ones_row[:], 1.0)

    psim = ctx.enter_context(tc.tile_pool(name="psim", bufs=2, space="PSUM"))
    pproj_cm = tc.tile_pool(name="pproj", bufs=2, space="PSUM")
    pproj = pproj_cm.__enter__()

    def qproj(half):
        for dc in range(4):
            p = pproj.tile([128, NS], F32, tag="proj")
            for kc in range(3):
                nc.tensor.matmul(
                    p[:],
                    s_wq[:, kc, dc * 128:(dc + 1) * 128],
                    s_xt[:, kc, half * NS:(half + 1) * NS],
                    start=(kc == 0), stop=(kc == 2),
                )
            nc.scalar.copy(s_qt[:, dc, half * NS:(half + 1) * NS], p[:])

    # ---- phase 0: just enough for the branch-0 max -> collective ----
    qproj(1)                                     # cond-half qT
    for dc in range(4):                          # branch-0 kT slice
        p = pproj.tile([128, 5 * L], F32, tag="proj")
        for kc in range(6):
            nc.tensor.matmul(
                p[0:128, 0:L],
                s_wk[:, kc, dc * 128:(dc + 1) * 128],
                s_ctxkt[:, kc, L:2 * L],
                start=(kc == 0), stop=(kc == 5),
            )
        nc.scalar.copy(s_ktc0[:, dc, :], p[0:128, 0:L])

    def qk0(h, psum_slice):
        nc.tensor.matmul(
            psum_slice,
            s_ktc0[(h % 2) * 64:(h % 2) * 64 + 64, h // 2, :],
            s_qt[(h % 2) * 64:(h % 2) * 64 + 64, h // 2, NS:NSB],
            start=True, stop=True,
        )

    for hp in range(4):
        p = psim.tile([L, 2, NS], F32, tag="sim")
        qk0(2 * hp, p[:, 0, :])
        qk0(2 * hp + 1, p[:, 1, :])
        nc.vector.reduce_max(out=s_lmax[:, 2 * hp:2 * hp + 2], in_=p[:],
                             axis=mybir.AxisListType.X)
        nc.scalar.copy(s_sc0[:, 2 * hp:2 * hp + 2, :], p[:])
    nc.vector.reduce_max(out=s_lm[:], in_=s_lmax[:], axis=mybir.AxisListType.X)
    nc.vector.tensor_scalar_mul(s_lm[:], s_lm[:], float(wdotw))

    cin = dram.tile([1, L], F32)
    cout = dram.tile([N_CORES, L], F32)
    nc.sync.dma_start(out=cin.rearrange("one f -> f one"), in_=s_lm[:])
    real_cc = nc.gpsimd.collective_compute(
        "AllGather", mybir.AluOpType.bypass,
        replica_groups=[list(range(N_CORES))],
        ins=[cin.opt()], outs=[cout.opt()],
    )

    # ---- remaining input DMA ----
    nc.sync.dma_start(out=s_wv[:], in_=d_wv.ap())
    nc.sync.dma_start(out=s_ctxvt[:], in_=d_ctxvt.ap())
    nc.sync.dma_start(out=s_wo[:], in_=d_wo.ap())
    nc.sync.dma_start(out=s_bo[:], in_=d_bo.ap())
    nc.sync.dma_start(out=s_id[:], in_=d_id.ap())
    nc.sync.dma_start(out=s_aet[:], in_=d_aet.ap().rearrange("h p f -> p h f"))

    # ---- phase 1: remaining projections ----
    qproj(0)                                     # uc-half qT
    for dc in range(4):                          # full kT (branch-0 cols unused)
        p = pproj.tile([128, 5 * L], F32, tag="proj")
        for kc in range(6):
            nc.tensor.matmul(
                p[:],
                s_wk[:, kc, dc * 128:(dc + 1) * 128],
                s_ctxkt[:, kc, :],
                start=(kc == 0), stop=(kc == 5),
            )
        nc.scalar.copy(s_kt[:, dc, :], p[:])
    for g in range(5):                           # v, with 1/C folded into cond
        p = pproj.tile([128, INNER], F32, tag="proj")
        for kc in range(6):
            nc.tensor.matmul(
                p[0:L, :],
                s_ctxvt[:, kc, g * L:(g + 1) * L],
                s_wv[:, kc, :],
                start=(kc == 0), stop=(kc == 5),
            )
        if g == G_UC:
            nc.scalar.copy(s_vp[:, g, :], p[0:L, :])
        else:
            nc.scalar.mul(s_vp[:, g, :], p[0:L, :], 1.0 / C)
    pproj_cm.__exit__(None, None, None)

    pzb = ctx.enter_context(tc.tile_pool(name="pzb", bufs=1, space="PSUM"))
    ppv = ctx.enter_context(tc.tile_pool(name="ppv", bufs=2, space="PSUM"))

    def qk(g, h, psum_slice):
        cols = slice(0, NS) if g == G_UC else slice(NS, NSB)
        nc.tensor.matmul(
            psum_slice,
            s_kt[(h % 2) * 64:(h % 2) * 64 + 64, h // 2, g * L:(g + 1) * L],
            s_qt[(h % 2) * 64:(h % 2) * 64 + 64, h // 2, cols],
            start=True, stop=True,
        )

    # ---- phase 3: uc + branches 1..3 (unit pairs) ----
    attn_c = {}
    anchors = {}

    def unit_pair(e_pair):
        zb = pzb.tile([L, 2, NS], F32, tag="zb")
        nc.tensor.matmul(zb[:, 0, :], ones77[:], e_pair[:, 0, :], start=True, stop=True)
        anchors["zb"] = nc.tensor.matmul(
            zb[:, 1, :], ones77[:], e_pair[:, 1, :], start=True, stop=True)
        rz = rzpool.tile([L, 2, NS], F32, tag="rz")
        nc.vector.reciprocal_approx_fast(out=rz[:], in_=zb[:])
        return rz

    for g in (0, 2, 3, 4):
        sims = []
        for hp in range(4):
            p = psim.tile([L, 2, NS], F32, tag="sim")
            qk(g, 2 * hp, p[:, 0, :])
            qk(g, 2 * hp + 1, p[:, 1, :])
            sims.append(p)
        for hp in range(4):
            e = epool.tile([L, 2, NS], F16, tag="e")
            anchors["exp"] = nc.scalar.activation(
                e[:], sims[hp][:], mybir.ActivationFunctionType.Exp)
            rz = unit_pair(e)
            for k in range(2):
                h = 2 * hp + k
                if g == G_UC:
                    pv = ppv.tile([64, NS], F32, tag="pv")
                    nc.tensor.matmul(pv[:], s_vp[:, 0, h * 64:(h + 1) * 64],
                                     e[:, k, :], start=True, stop=True)
                    nc.vector.tensor_mul(
                        s_om[(h % 2) * 64:(h % 2) * 64 + 64, h // 2, 0:NS],
                        pv[:], rz[0:64, k, :])
                else:
                    a = apool.tile([L, NS], F16, tag="attn")
                    if g == 3:
                        # offload one branch's normalization to the idle gpsimd
                        mi = nc.gpsimd.tensor_mul(a[:], e[:, k, :], rz[:, k, :])
                        tile.add_dep_helper(mi.ins, real_cc.ins, sync=False,
                                            reason="gpsimd mul after collective")
                    else:
                        anchors["mul"] = nc.vector.tensor_mul(a[:], e[:, k, :],
                                                              rz[:, k, :])
                    attn_c[(g, h)] = a

    # ---- pre-accumulate the cond PV partials for branches 1..3 ----
    for h in range(H):
        pv = ppv.tile([64, NS], F32, tag="pv")
        for i, g in enumerate((2, 3, 4)):
            nc.tensor.matmul(pv[:], s_vp[:, g, h * 64:(h + 1) * 64],
                             attn_c.pop((g, h))[:], start=(i == 0), stop=(i == 2))
        anchors["evac"] = nc.vector.tensor_copy(
            s_pvp[(h % 2) * 64:(h % 2) * 64 + 64, h // 2, :], pv[:])

    # ---- uc half of the output projection (independent of branch 0) ----
    def wo_half(half, pool):
        for oc in range(3):
            ow = 128 if oc < 2 else 64
            p = pool.tile([128, NS], F32, tag="pv")
            for kc in range(4):
                nc.tensor.matmul(
                    p[0:ow, :],
                    s_wo[:, kc, oc * 128:oc * 128 + ow],
                    s_om[:, kc, half * NS:(half + 1) * NS],
                    start=(kc == 0), stop=(kc == 3),
                )
            nc.scalar.add(s_y[0:ow, oc, half * NS:(half + 1) * NS], p[0:ow, :],
                          s_bo[0:ow, oc:oc + 1])
        for oc in range(3):
            ow = 128 if oc < 2 else 64
            nc.sync.dma_start(
                out=d_yt.ap()[oc * 128:oc * 128 + ow, half * NS:(half + 1) * NS],
                in_=s_y[0:ow, oc, half * NS:(half + 1) * NS])

    wo_half(0, ppv)

    # ---- phase 4: wmask from the gathered maxima, branch 0, PV finish ----
    nc.sync.dma_start(out=s_maxrow8[:], in_=cout.rearrange("r f -> (r f)"))
    red = nc.vector.reduce_max(out=s_wm[:], in_=s_maxrow8[:], axis=mybir.AxisListType.X)
    tile.add_dep_helper(red.ins, anchors["mul"].ins, sync=False,
                        reason="defer wmask path behind group work")
    p_wm = pzb.tile([L, 2, NS], F32, tag="zb")
    bc = nc.tensor.matmul(p_wm[:, 0, 0:1], ones_row[0:1, 0:L], s_wm[:],
                          start=True, stop=True)
    tile.add_dep_helper(bc.ins, anchors["zb"].ins, sync=False,
                        reason="defer wmask bcast behind group matmuls")
    nc.vector.tensor_copy(s_wmcol[:], p_wm[:, 0, 0:1])

    first_p4_exp = None
    for hp in range(4):
        msk = work.tile([L, 2, NS], F32, tag="msk")
        nc.vector.scalar_tensor_tensor(
            out=msk[:], in0=s_aet[:, 2 * hp:2 * hp + 2, :], scalar=s_wmcol[:],
            in1=s_sc0[:, 2 * hp:2 * hp + 2, :],
            op0=mybir.AluOpType.mult, op1=mybir.AluOpType.add,
        )
        e = epool.tile([L, 2, NS], F16, tag="e")
        ei = nc.scalar.activation(e[:], msk[:], mybir.ActivationFunctionType.Exp)
        if first_p4_exp is None:
            first_p4_exp = ei
            tile.add_dep_helper(ei.ins, anchors["exp"].ins, sync=False,
                                reason="defer branch-0 exp behind group exps")
        rz = unit_pair(e)
        for k in range(2):
            h = 2 * hp + k
            a = apool.tile([L, NS], F16, tag="attn")
            nc.vector.tensor_mul(a[:], e[:, k, :], rz[:, k, :])
            rows = slice((h % 2) * 64, (h % 2) * 64 + 64)
            pv = ppv.tile([64, NS], F32, tag="pv")
            nc.tensor.matmul(pv[:], s_vp[:, 1, h * 64:(h + 1) * 64],
                             a[:], start=True, stop=False)
            # merge the parked branch 1..3 partial via identity accumulate
            nc.tensor.matmul(pv[:], s_id[rows, :], s_pvp[rows, h // 2, :],
                             start=False, stop=True)
            nc.scalar.copy(s_om[rows, h // 2, NS:NSB], pv[:])

    # ---- phase 5: cond half of the output projection ----
    wo_half(1, ppv)
    ctx.pop_all().close()


_CACHE = {}


def _packP(a, p=128):
    rows, cols = a.shape
    pad = (-rows) % p
    if pad:
        a = np.vstack([a, np.zeros((pad, cols), a.dtype)])
    c = a.shape[0] // p
    return np.ascontiguousarray(a.reshape(c, p, cols).transpose(1, 0, 2))


def kernel(x, uc_context, ck, cv, attn_extra, Wq, Wk, Wv, Wo, bo, t):
    global LAST_RESULTS
    x = np.ascontiguousarray(np.asarray(x, np.float32))
    uc_context = np.asarray(uc_context, np.float32)
    ck = np.asarray(ck, np.float32)
    cv = np.asarray(cv, np.float32)
    attn_extra = np.asarray(attn_extra, np.float32)
    Wq = np.asarray(Wq, np.float32)
    Wk = np.asarray(Wk, np.float32)
    Wv = np.asarray(Wv, np.float32)
    Wo = np.asarray(Wo, np.float32)
    bo = np.asarray(bo, np.float32)
    tv = float(np.asarray(t))
    wdotw = W_DOT * (tv / TOTAL_STEP) * SCHED

    if wdotw not in _CACHE:
        _CACHE[wdotw] = build_kernel(wdotw)
    nc = _CACHE[wdotw]

    # host-side input prep (layout only)
    wq_pad = np.zeros((384, INNER), np.float16)
    wq_pad[:DQ] = (Wq * SCALE).astype(np.float16)
    bo_pad = np.zeros((384,), np.float32)
    bo_pad[:DQ] = bo
    wqp = _packP(wq_pad)
    wkp = _packP(Wk.astype(np.float16))
    wvp = _packP(Wv.astype(np.float16))
    wop = _packP(Wo.astype(np.float16))
    bop = np.ascontiguousarray(bo_pad.reshape(3, 128).T)
    ctxK = np.concatenate([uc_context[0][None], ck[:, 0]], axis=0)  # [5, 77, 768]
    ctxV = np.concatenate([uc_context[0][None], cv[:, 0]], axis=0)
    ctxkt = _packP(np.ascontiguousarray(
        ctxK.transpose(2, 0, 1).reshape(DC, 5 * L)).astype(np.float16))
    ctxvt = _packP(np.ascontiguousarray(
        ctxV.transpose(2, 0, 1).reshape(DC, 5 * L)).astype(np.float16))
    idm = np.ascontiguousarray(np.vstack([np.eye(64, dtype=np.float16)] * 2))

    in_maps = []
    for c in range(N_CORES):
        rows = slice(c * NS, (c + 1) * NS)
        xt = np.zeros((384, NSB), np.float16)
        xt[:DQ, :NS] = x[0, rows].T.astype(np.float16)
        xt[:DQ, NS:] = x[1, rows].T.astype(np.float16)
        aet = np.ascontiguousarray(attn_extra[:, rows, :].transpose(0, 2, 1))
        in_maps.append({
            "xt": _packP(xt), "wq": wqp, "wk": wkp, "wv": wvp, "wo": wop,
            "bo": bop, "ctxkt": ctxkt, "ctxvt": ctxvt, "aet": aet, "idm": idm,
        })

    import os as _os
    _tc = None
    if _os.environ.get("KERNEL_TRACE_ALL") == "1":
        _tc = list(range(N_CORES))
    res = bass_utils.run_bass_kernel_spmd(
        nc, in_maps, core_ids=list(range(N_CORES)), trace=TRACE, trace_cores=_tc,
    )
    LAST_RESULTS = res

    out = np.empty((2, N, DQ), np.float32)
    for c in range(N_CORES):
        rows = slice(c * NS, (c + 1) * NS)
        yt = res.results[c]["yt"]
        out[0, rows] = yt[:, :NS].T.astype(np.float32)
        out[1, rows] = yt[:, NS:].T.astype(np.float32)
    return out
